# revision 1
# baseline (speedup 1.0000x reference)
"""Trainium2 Bass kernel for quantized Conv2d(3x3) + BatchNorm + PACT.

Reference computation:
  wq = DoReFa-quantized weight (4-bit)  -> wq = (2*round(t15)-15)/15 where
       t15 = (tanh(w)/(2*max|tanh(w)|)+0.5)*15, so 15*wq is an exact odd
       integer in [-15, 15] (exactly representable in bf16).
  y  = conv2d(x, wq, stride 1, pad 1)          (B,Cout,H,W)
  y  = y*inv + (beta - mean*inv)               inv = gamma/sqrt(var+eps)
  out= round(clip(y,0,alpha)*15/alpha)*(alpha/15)

Strategy (8 NeuronCores, data-parallel over batch, 4 images/core):
  - Host: quantize weights to integer bf16 (exact), fold BN+PACT scales,
    split x into bf16 hi+lo halves (hi+lo ~= fp32 accurate), zero-pad each
    image to 58x58 so shifted conv windows never wrap.
  - Device: conv as 9 shifted matmuls x 2 C_in K-tiles x 2 (hi/lo) passes,
    all 36 accumulating into one PSUM tile [128 Cout x (8 rows x 56 cols)].
    Epilogue on DVE: BN scale/bias, round-to-nearest-even via the
    +/-1.5*2^23 magic constant, clip to [0,15], scale by alpha/15.
"""

import numpy as np
import ml_dtypes

B, C_IN, C_OUT, H, W = 32, 256, 256, 56, 56
N_CORES = 8
B_LOC = B // N_CORES          # 4 images per core
HP, WP = H + 2, W + 2         # 58x58 padded
KT = C_IN // 128              # 2 K-tiles of 128 input channels
MT = C_OUT // 128             # 2 M-tiles of 128 output channels
ROWS = 8                      # output rows per PSUM tile (N = 8*56 = 448)
MAGIC = 12582912.0            # 1.5 * 2**23: fp32 round-to-nearest-even

_BF16 = ml_dtypes.bfloat16
_CACHE = {}


def _build_nc():
    import concourse.bacc as bacc
    import concourse.mybir as mybir
    import concourse.tile as tile

    f32 = mybir.dt.float32
    bf16 = mybir.dt.bfloat16
    op = mybir.AluOpType

    nc = bacc.Bacc("TRN2", target_bir_lowering=False, num_devices=N_CORES)

    xh_d = nc.dram_tensor("xh", [KT, 128, B_LOC, HP, WP], bf16, kind="ExternalInput").ap()
    xl_d = nc.dram_tensor("xl", [KT, 128, B_LOC, HP, WP], bf16, kind="ExternalInput").ap()
    wq_d = nc.dram_tensor("wq", [KT, 128, 9, C_OUT], bf16, kind="ExternalInput").ap()
    cs_d = nc.dram_tensor("cs", [128, MT, 4], f32, kind="ExternalInput").ap()
    out_d = nc.dram_tensor("out", [B_LOC, MT, 128, H, W], f32, kind="ExternalOutput").ap()

    with tile.TileContext(nc) as tc:
        with (
            tc.tile_pool(name="consts", bufs=1) as cpool,
            tc.tile_pool(name="xres", bufs=1) as xpool,
            tc.tile_pool(name="psum", bufs=4, space="PSUM") as ppool,
            tc.tile_pool(name="work", bufs=4) as wpool,
            tc.tile_pool(name="outs", bufs=4) as opool,
        ):
            w_sb = cpool.tile([128, KT, 9, C_OUT], bf16, name="w_sb")
            for kt in range(KT):
                nc.sync.dma_start(out=w_sb[:, kt], in_=wq_d[kt])
            cs_sb = cpool.tile([128, MT, 4], f32, name="cs_sb")
            nc.sync.dma_start(out=cs_sb[:, :, :], in_=cs_d[:, :, :])

            # Resident padded x, bf16 hi/lo, one tile per (pass, ktile, image)
            # so matmuls for image b only wait on image b's DMA.
            xt = {}
            for p, src in ((0, xh_d), (1, xl_d)):
                for kt in range(KT):
                    for b in range(B_LOC):
                        t = xpool.tile([128, HP, WP], bf16, name=f"x_{p}_{kt}_{b}")
                        nc.sync.dma_start(out=t[:, :, :], in_=src[kt, :, b])
                        xt[(p, kt, b)] = t

            n_acc = 2 * KT * 9
            for b in range(B_LOC):
                for m in range(MT):
                    for h0 in range(0, H, ROWS):
                        ps = ppool.tile([128, ROWS, W], f32, name="ps", tag="ps")
                        idx = 0
                        for p in range(2):
                            for kt in range(KT):
                                xs = xt[(p, kt, b)]
                                for dh in (-1, 0, 1):
                                    for dw in (-1, 0, 1):
                                        o = (dh + 1) * 3 + (dw + 1)
                                        nc.tensor.matmul(
                                            ps[:, :, :],
                                            lhsT=w_sb[:, kt, o, m * 128:(m + 1) * 128],
                                            rhs=xs[:, h0 + dh + 1:h0 + dh + 1 + ROWS,
                                                   dw + 1:dw + 1 + W],
                                            start=(idx == 0),
                                            stop=(idx == n_acc - 1),
                                        )
                                        idx += 1
                        # epilogue: z = ps*scale + bias ; r = rne(z) ;
                        # rc = clip(r,0,15) ; out = rc*(alpha/15)
                        z = wpool.tile([128, ROWS, W], f32, name="z", tag="z")
                        nc.vector.tensor_scalar(
                            z[:, :, :], ps[:, :, :],
                            cs_sb[:, m, 0:1], cs_sb[:, m, 1:2], op.mult, op.add)
                        r = wpool.tile([128, ROWS, W], f32, name="r", tag="r")
                        nc.vector.tensor_scalar(
                            r[:, :, :], z[:, :, :], MAGIC, MAGIC, op.add, op.subtract)
                        rc = wpool.tile([128, ROWS, W], f32, name="rc", tag="rc")
                        nc.vector.tensor_scalar(
                            rc[:, :, :], r[:, :, :], 0.0, 15.0, op.max, op.min)
                        ot = opool.tile([128, ROWS, W], f32, name="ot", tag="ot")
                        nc.vector.tensor_scalar(
                            ot[:, :, :], rc[:, :, :], cs_sb[:, m, 2:3], None, op.mult)
                        nc.sync.dma_start(
                            out=out_d[b, m, :, h0:h0 + ROWS, :], in_=ot[:, :, :])

    nc.compile()
    return nc


def _get_nc():
    if "nc" not in _CACHE:
        _CACHE["nc"] = _build_nc()
    return _CACHE["nc"]


def _prep_inputs(x, weight, gamma, beta, running_mean, running_var, alpha):
    # --- weight quantization, mirroring the fp32 reference ops exactly ---
    w32 = np.asarray(weight, dtype=np.float32)
    t = np.tanh(w32)
    t = t / (np.float32(2.0) * np.max(np.abs(t))) + np.float32(0.5)
    q = np.round(t * np.float32(15.0))          # integers 0..15 (fp32 exact)
    w_int = 2.0 * q.astype(np.float64) - 15.0   # odd integers -15..15
    # [Cout,Cin,3,3] -> [Cin,3*3,Cout] -> [KT,128,9,Cout], exact in bf16
    wq = np.ascontiguousarray(w_int.transpose(1, 2, 3, 0)).reshape(
        C_IN, 9, C_OUT).reshape(KT, 128, 9, C_OUT).astype(_BF16)

    # --- BN + PACT constant folding (reference fp32 semantics first) ---
    g = np.asarray(gamma, dtype=np.float32)
    bt = np.asarray(beta, dtype=np.float32)
    rm = np.asarray(running_mean, dtype=np.float32)
    rv = np.asarray(running_var, dtype=np.float32)
    a = np.float32(np.asarray(alpha).reshape(-1)[0])
    inv = g / np.sqrt(rv + np.float32(1e-5))
    bb = bt - rm * inv
    # psum = 15*conv(x,wq); z = (psum/15*inv + bb)*(15/a) = psum*(inv/a) + bb*(15/a)
    scale_c = (inv.astype(np.float64) / np.float64(a)).astype(np.float32)
    bias_c = (bb.astype(np.float64) * (15.0 / np.float64(a))).astype(np.float32)
    fscale = np.full(C_OUT, a / np.float32(15.0), dtype=np.float32)
    cs = np.zeros((128, MT, 4), dtype=np.float32)
    for m in range(MT):
        cs[:, m, 0] = scale_c[m * 128:(m + 1) * 128]
        cs[:, m, 1] = bias_c[m * 128:(m + 1) * 128]
        cs[:, m, 2] = fscale[m * 128:(m + 1) * 128]

    # --- x: channel-major, zero-padded halo, bf16 hi/lo split ---
    x32 = np.asarray(x, dtype=np.float32)
    xp = np.zeros((C_IN, B, HP, WP), dtype=np.float32)
    xp[:, :, 1:1 + H, 1:1 + W] = x32.transpose(1, 0, 2, 3)
    xh = xp.astype(_BF16)
    xl = (xp - xh.astype(np.float32)).astype(_BF16)
    xh = xh.reshape(KT, 128, B, HP, WP)
    xl = xl.reshape(KT, 128, B, HP, WP)

    in_maps = []
    for i in range(N_CORES):
        sl = slice(i * B_LOC, (i + 1) * B_LOC)
        in_maps.append({
            "xh": np.ascontiguousarray(xh[:, :, sl]),
            "xl": np.ascontiguousarray(xl[:, :, sl]),
            "wq": wq,
            "cs": cs,
        })
    return in_maps


def kernel(x, weight, gamma, beta, running_mean, running_var, alpha):
    from concourse.bass_utils import run_bass_kernel_spmd

    nc = _get_nc()
    in_maps = _prep_inputs(x, weight, gamma, beta, running_mean, running_var, alpha)
    res = run_bass_kernel_spmd(nc, in_maps, list(range(N_CORES)))
    outs = [r["out"].reshape(B_LOC, C_OUT, H, W) for r in res.results]
    return np.concatenate(outs, axis=0)


# revision 5
# speedup vs baseline: 37.7889x; 37.7889x over previous
"""Trainium2 Bass kernel for quantized Conv2d(3x3) + BatchNorm + PACT.

Reference computation:
  wq = DoReFa-quantized weight (4-bit)  -> wq = (2*round(t15)-15)/15 where
       t15 = (tanh(w)/(2*max|tanh(w)|)+0.5)*15, so 15*wq is an exact odd
       integer in [-15, 15] (exactly representable in bf16).
  y  = conv2d(x, wq, stride 1, pad 1)          (B,Cout,H,W)
  y  = y*inv + (beta - mean*inv)               inv = gamma/sqrt(var+eps)
  out= round(clip(y,0,alpha)*15/alpha)*(alpha/15)

Strategy (8 NeuronCores, data-parallel over batch, 4 images/core):
  - Host: quantize weights to integer bf16 (exact), fold BN+PACT scales,
    split x into bf16 hi+lo halves (hi+lo ~= fp32 accurate), zero-pad each
    image to 58x58 so shifted conv windows never wrap.
  - Device: conv as 9 shifted matmuls x 2 C_in K-tiles x 2 (hi/lo) passes,
    all 36 accumulating into one PSUM tile [128 Cout x (8 rows x 56 cols)].
    Epilogue on DVE: BN scale/bias, round-to-nearest-even via the
    +/-1.5*2^23 magic constant, clip to [0,15], scale by alpha/15.
"""

import numpy as np
import ml_dtypes

B, C_IN, C_OUT, H, W = 32, 256, 256, 56, 56
N_CORES = 8
B_LOC = B // N_CORES          # 4 images per core
HP, WP = H + 2, W + 2         # 58x58 padded
KT = C_IN // 128              # 2 K-tiles of 128 input channels
MT = C_OUT // 128             # 2 M-tiles of 128 output channels
ROWS = 8                      # output rows per PSUM tile (N = 8*56 = 448)
MAGIC = 12582912.0            # 1.5 * 2**23: fp32 round-to-nearest-even

_BF16 = ml_dtypes.bfloat16
_CACHE = {}


def _build_nc(reps=1):
    import concourse.bacc as bacc
    import concourse.mybir as mybir
    import concourse.tile as tile

    f32 = mybir.dt.float32
    bf16 = mybir.dt.bfloat16
    op = mybir.AluOpType

    nc = bacc.Bacc("TRN2", target_bir_lowering=False, num_devices=N_CORES)

    xh_d = nc.dram_tensor("xh", [KT, 128, B_LOC, HP, WP], bf16, kind="ExternalInput").ap()
    xl_d = nc.dram_tensor("xl", [KT, 128, B_LOC, HP, WP], bf16, kind="ExternalInput").ap()
    wq_d = nc.dram_tensor("wq", [KT, 128, 9, C_OUT], bf16, kind="ExternalInput").ap()
    cs_d = nc.dram_tensor("cs", [128, MT, 4], f32, kind="ExternalInput").ap()
    out_d = nc.dram_tensor("out", [B_LOC, MT, 128, H, W], f32, kind="ExternalOutput").ap()

    with tile.TileContext(nc) as tc:
        with (
            tc.tile_pool(name="consts", bufs=1) as cpool,
            tc.tile_pool(name="xres", bufs=1) as xpool,
            tc.tile_pool(name="psum", bufs=4, space="PSUM") as ppool,
            tc.tile_pool(name="work", bufs=4) as wpool,
            tc.tile_pool(name="outs", bufs=4) as opool,
        ):
            w_sb = cpool.tile([128, KT, 9, C_OUT], bf16, name="w_sb")
            for kt in range(KT):
                nc.sync.dma_start(out=w_sb[:, kt], in_=wq_d[kt])
            cs_sb = cpool.tile([128, MT, 4], f32, name="cs_sb")
            nc.sync.dma_start(out=cs_sb[:, :, :], in_=cs_d[:, :, :])

            # Resident padded x, bf16 hi/lo, one tile per (pass, ktile, image)
            # so matmuls for image b only wait on image b's DMA.
            xt = {}
            for p, src in ((0, xh_d), (1, xl_d)):
                for kt in range(KT):
                    for b in range(B_LOC):
                        t = xpool.tile([128, HP, WP], bf16, name=f"x_{p}_{kt}_{b}")
                        nc.sync.dma_start(out=t[:, :, :], in_=src[kt, :, b])
                        xt[(p, kt, b)] = t

            n_acc = 2 * KT * 9
            for _rep, b, m in [(r, b, m) for r in range(reps)
                               for b in range(B_LOC) for m in range(MT)]:
                    for h0 in range(0, H, ROWS):
                        ps = ppool.tile([128, ROWS, W], f32, name="ps", tag="ps")
                        idx = 0
                        for p in range(2):
                            for kt in range(KT):
                                xs = xt[(p, kt, b)]
                                for dh in (-1, 0, 1):
                                    for dw in (-1, 0, 1):
                                        o = (dh + 1) * 3 + (dw + 1)
                                        nc.tensor.matmul(
                                            ps[:, :, :],
                                            lhsT=w_sb[:, kt, o, m * 128:(m + 1) * 128],
                                            rhs=xs[:, h0 + dh + 1:h0 + dh + 1 + ROWS,
                                                   dw + 1:dw + 1 + W],
                                            start=(idx == 0),
                                            stop=(idx == n_acc - 1),
                                        )
                                        idx += 1
                        # epilogue: z = ps*scale + bias ; r = rne(z) ;
                        # rc = clip(r,0,15) ; out = rc*(alpha/15)
                        z = wpool.tile([128, ROWS, W], f32, name="z", tag="z")
                        nc.vector.tensor_scalar(
                            z[:, :, :], ps[:, :, :],
                            cs_sb[:, m, 0:1], cs_sb[:, m, 1:2], op.mult, op.add)
                        r = wpool.tile([128, ROWS, W], f32, name="r", tag="r")
                        nc.vector.tensor_scalar(
                            r[:, :, :], z[:, :, :], MAGIC, MAGIC, op.add, op.subtract)
                        rc = wpool.tile([128, ROWS, W], f32, name="rc", tag="rc")
                        nc.vector.tensor_scalar(
                            rc[:, :, :], r[:, :, :], 0.0, 15.0, op.max, op.min)
                        ot = opool.tile([128, ROWS, W], f32, name="ot", tag="ot")
                        nc.vector.tensor_scalar(
                            ot[:, :, :], rc[:, :, :], cs_sb[:, m, 2:3], None, op.mult)
                        nc.sync.dma_start(
                            out=out_d[b, m, :, h0:h0 + ROWS, :], in_=ot[:, :, :])

    nc.compile()
    return nc


def _get_nc(reps=1):
    key = ("nc", reps)
    if key not in _CACHE:
        _CACHE[key] = _build_nc(reps)
    return _CACHE[key]


def _prep_inputs(x, weight, gamma, beta, running_mean, running_var, alpha):
    # --- weight quantization, mirroring the fp32 reference ops exactly ---
    w32 = np.asarray(weight, dtype=np.float32)
    t = np.tanh(w32)
    t = t / (np.float32(2.0) * np.max(np.abs(t))) + np.float32(0.5)
    q = np.round(t * np.float32(15.0))          # integers 0..15 (fp32 exact)
    w_int = 2.0 * q.astype(np.float64) - 15.0   # odd integers -15..15
    # [Cout,Cin,3,3] -> [Cin,3*3,Cout] -> [KT,128,9,Cout], exact in bf16
    wq = np.ascontiguousarray(w_int.transpose(1, 2, 3, 0)).reshape(
        C_IN, 9, C_OUT).reshape(KT, 128, 9, C_OUT).astype(_BF16)

    # --- BN + PACT constant folding (reference fp32 semantics first) ---
    g = np.asarray(gamma, dtype=np.float32)
    bt = np.asarray(beta, dtype=np.float32)
    rm = np.asarray(running_mean, dtype=np.float32)
    rv = np.asarray(running_var, dtype=np.float32)
    a = np.float32(np.asarray(alpha).reshape(-1)[0])
    inv = g / np.sqrt(rv + np.float32(1e-5))
    bb = bt - rm * inv
    # psum = 15*conv(x,wq); z = (psum/15*inv + bb)*(15/a) = psum*(inv/a) + bb*(15/a)
    scale_c = (inv.astype(np.float64) / np.float64(a)).astype(np.float32)
    bias_c = (bb.astype(np.float64) * (15.0 / np.float64(a))).astype(np.float32)
    fscale = np.full(C_OUT, a / np.float32(15.0), dtype=np.float32)
    cs = np.zeros((128, MT, 4), dtype=np.float32)
    for m in range(MT):
        cs[:, m, 0] = scale_c[m * 128:(m + 1) * 128]
        cs[:, m, 1] = bias_c[m * 128:(m + 1) * 128]
        cs[:, m, 2] = fscale[m * 128:(m + 1) * 128]

    # --- x: channel-major, zero-padded halo, bf16 hi/lo split ---
    x32 = np.asarray(x, dtype=np.float32)
    xp = np.zeros((C_IN, B, HP, WP), dtype=np.float32)
    xp[:, :, 1:1 + H, 1:1 + W] = x32.transpose(1, 0, 2, 3)
    xh = xp.astype(_BF16)
    xl = (xp - xh.astype(np.float32)).astype(_BF16)
    xh = xh.reshape(KT, 128, B, HP, WP)
    xl = xl.reshape(KT, 128, B, HP, WP)

    in_maps = []
    for i in range(N_CORES):
        sl = slice(i * B_LOC, (i + 1) * B_LOC)
        in_maps.append({
            "xh": np.ascontiguousarray(xh[:, :, sl]),
            "xl": np.ascontiguousarray(xl[:, :, sl]),
            "wq": wq,
            "cs": cs,
        })
    return in_maps


def kernel(x, weight, gamma, beta, running_mean, running_var, alpha):
    from concourse.bass_utils import run_bass_kernel_spmd

    nc = _get_nc()
    in_maps = _prep_inputs(x, weight, gamma, beta, running_mean, running_var, alpha)
    res = run_bass_kernel_spmd(nc, in_maps, list(range(N_CORES)))
    outs = [r["out"].reshape(B_LOC, C_OUT, H, W) for r in res.results]
    return np.concatenate(outs, axis=0)


# revision 6
# speedup vs baseline: 106.7333x; 2.8245x over previous
"""Trainium2 Bass kernel for quantized Conv2d(3x3) + BatchNorm + PACT.

Reference computation:
  wq = DoReFa-quantized weight (4-bit)  -> wq = (2*round(t15)-15)/15 where
       t15 = (tanh(w)/(2*max|tanh(w)|)+0.5)*15, so 15*wq is an exact odd
       integer in [-15, 15] (exactly representable in bf16/fp8-e4m3).
  y  = conv2d(x, wq, stride 1, pad 1)          (B,Cout,H,W)
  y  = y*inv + (beta - mean*inv)               inv = gamma/sqrt(var+eps)
  out= round(clip(y,0,alpha)*15/alpha)*(alpha/15)

Strategy (8 NeuronCores, data-parallel over batch, 4 images/core):
  - Host: quantize weights to integer values (exact in bf16), fold BN+PACT
    scales, split x = hi + lo with hi = bf16(x) and lo = the fp32 residual:
      hi-pass: bf16 matmuls (weights 15*wq exact ints)
      lo-pass: fp8 e4m3 with DoubleRow perf mode (K=256 per matmul, 2x rate);
               lo scaled by 2^8, lo-weights scaled by 2^-8 (both exact), so
               the product lands in the same PSUM accumulation unscaled.
    x tensors are zero-padded per image to 58x58 so shifted windows never wrap.
  - Device: conv as 9 shifted matmuls: per PSUM tile [128 Cout x (8 rows x 56
    cols)]: 9 offsets x 2 C_in K-tiles bf16 (hi) + 9 offsets DoubleRow fp8
    (lo) = 27 matmuls accumulating into one PSUM bank.
    Epilogue on DVE: BN scale/bias, round-to-nearest-even via the +/-1.5*2^23
    magic constant, clip to [0,15], scale by alpha/15.
"""

import numpy as np
import ml_dtypes

B, C_IN, C_OUT, H, W = 32, 256, 256, 56, 56
N_CORES = 8
B_LOC = B // N_CORES          # 4 images per core
HP, WP = H + 2, W + 2         # 58x58 padded
KT = C_IN // 128              # 2 K-tiles of 128 input channels
MT = C_OUT // 128             # 2 M-tiles of 128 output channels
ROWS = 8                      # output rows per PSUM tile (N = 8*56 = 448)
MAGIC = 12582912.0            # 1.5 * 2**23: fp32 round-to-nearest-even
LO_SHIFT = 8                  # lo-pass scaling: x_lo*2^8, w*2^-8 (both exact)

_BF16 = ml_dtypes.bfloat16
_E4M3 = ml_dtypes.float8_e4m3
_CACHE = {}


def _build_nc(reps=1):
    import concourse.bacc as bacc
    import concourse.mybir as mybir
    import concourse.tile as tile

    f32 = mybir.dt.float32
    bf16 = mybir.dt.bfloat16
    fp8 = mybir.dt.float8e4
    op = mybir.AluOpType

    nc = bacc.Bacc("TRN2", target_bir_lowering=False, num_devices=N_CORES)

    xh_d = nc.dram_tensor("xh", [KT, 128, B_LOC, HP, WP], bf16, kind="ExternalInput").ap()
    xl_d = nc.dram_tensor("xl", [128, 2, B_LOC, HP, WP], fp8, kind="ExternalInput").ap()
    wq_d = nc.dram_tensor("wq", [KT, 128, 9, C_OUT], bf16, kind="ExternalInput").ap()
    w8_d = nc.dram_tensor("w8", [128, 2, 9, C_OUT], fp8, kind="ExternalInput").ap()
    cs_d = nc.dram_tensor("cs", [128, MT, 4], f32, kind="ExternalInput").ap()
    out_d = nc.dram_tensor("out", [B_LOC, MT, 128, H, W], f32, kind="ExternalOutput").ap()

    with tile.TileContext(nc) as tc:
        with (
            tc.tile_pool(name="consts", bufs=1) as cpool,
            tc.tile_pool(name="xres", bufs=1) as xpool,
            tc.tile_pool(name="psum", bufs=4, space="PSUM") as ppool,
            tc.tile_pool(name="work", bufs=4) as wpool,
            tc.tile_pool(name="outs", bufs=4) as opool,
        ):
            w_sb = cpool.tile([128, KT, 9, C_OUT], bf16, name="w_sb")
            for kt in range(KT):
                nc.sync.dma_start(out=w_sb[:, kt], in_=wq_d[kt])
            w8_sb = cpool.tile([128, 2, 9, C_OUT], fp8, name="w8_sb")
            nc.sync.dma_start(out=w8_sb[:, :, :, :], in_=w8_d[:, :, :, :])
            cs_sb = cpool.tile([128, MT, 4], f32, name="cs_sb")
            nc.sync.dma_start(out=cs_sb[:, :, :], in_=cs_d[:, :, :])

            # Resident padded x: bf16 hi per (ktile, image); fp8 lo per image
            # (both K-tiles packed in the DoubleRow pair dim).
            xh_t, xl_t = {}, {}
            for kt in range(KT):
                for b in range(B_LOC):
                    t = xpool.tile([128, HP, WP], bf16, name=f"xh_{kt}_{b}")
                    nc.sync.dma_start(out=t[:, :, :], in_=xh_d[kt, :, b])
                    xh_t[(kt, b)] = t
            for b in range(B_LOC):
                t = xpool.tile([128, 2, HP, WP], fp8, name=f"xl_{b}")
                nc.sync.dma_start(out=t[:, :, :, :], in_=xl_d[:, :, b])
                xl_t[b] = t

            n_acc = KT * 9 + 9
            for _rep, b, m in [(r, b, m) for r in range(reps)
                               for b in range(B_LOC) for m in range(MT)]:
                    for h0 in range(0, H, ROWS):
                        ps = ppool.tile([128, ROWS, W], f32, name="ps", tag="ps")
                        idx = 0
                        for kt in range(KT):
                            xs = xh_t[(kt, b)]
                            for dh in (-1, 0, 1):
                                for dw in (-1, 0, 1):
                                    o = (dh + 1) * 3 + (dw + 1)
                                    nc.tensor.matmul(
                                        ps[:, :, :],
                                        lhsT=w_sb[:, kt, o, m * 128:(m + 1) * 128],
                                        rhs=xs[:, h0 + dh + 1:h0 + dh + 1 + ROWS,
                                               dw + 1:dw + 1 + W],
                                        start=(idx == 0), stop=False,
                                    )
                                    idx += 1
                        x8 = xl_t[b]
                        for dh in (-1, 0, 1):
                            for dw in (-1, 0, 1):
                                o = (dh + 1) * 3 + (dw + 1)
                                nc.tensor.matmul(
                                    ps[:, :, :],
                                    lhsT=w8_sb[:, :, o, m * 128:(m + 1) * 128],
                                    rhs=x8[:, :, h0 + dh + 1:h0 + dh + 1 + ROWS,
                                           dw + 1:dw + 1 + W],
                                    start=False, stop=(idx == n_acc - 1),
                                    perf_mode=mybir.MatmulPerfMode.DoubleRow,
                                )
                                idx += 1
                        # epilogue: z = ps*scale + bias ; r = rne(z) ;
                        # rc = clip(r,0,15) ; out = rc*(alpha/15)
                        z = wpool.tile([128, ROWS, W], f32, name="z", tag="z")
                        nc.vector.tensor_scalar(
                            z[:, :, :], ps[:, :, :],
                            cs_sb[:, m, 0:1], cs_sb[:, m, 1:2], op.mult, op.add)
                        r = wpool.tile([128, ROWS, W], f32, name="r", tag="r")
                        nc.vector.tensor_scalar(
                            r[:, :, :], z[:, :, :], MAGIC, MAGIC, op.add, op.subtract)
                        rc = wpool.tile([128, ROWS, W], f32, name="rc", tag="rc")
                        nc.vector.tensor_scalar(
                            rc[:, :, :], r[:, :, :], 0.0, 15.0, op.max, op.min)
                        ot = opool.tile([128, ROWS, W], f32, name="ot", tag="ot")
                        nc.vector.tensor_scalar(
                            ot[:, :, :], rc[:, :, :], cs_sb[:, m, 2:3], None, op.mult)
                        nc.sync.dma_start(
                            out=out_d[b, m, :, h0:h0 + ROWS, :], in_=ot[:, :, :])

    nc.compile()
    return nc


def _get_nc(reps=1):
    key = ("nc", reps)
    if key not in _CACHE:
        _CACHE[key] = _build_nc(reps)
    return _CACHE[key]


def _prep_inputs(x, weight, gamma, beta, running_mean, running_var, alpha):
    # --- weight quantization, mirroring the fp32 reference ops exactly ---
    w32 = np.asarray(weight, dtype=np.float32)
    t = np.tanh(w32)
    t = t / (np.float32(2.0) * np.max(np.abs(t))) + np.float32(0.5)
    q = np.round(t * np.float32(15.0))          # integers 0..15 (fp32 exact)
    w_int = 2.0 * q.astype(np.float64) - 15.0   # odd integers -15..15
    # [Cout,Cin,3,3] -> [Cin,9,Cout]; exact in bf16
    w_t = np.ascontiguousarray(w_int.transpose(1, 2, 3, 0)).reshape(C_IN, 9, C_OUT)
    wq = w_t.reshape(KT, 128, 9, C_OUT).astype(_BF16)
    # lo-pass weights: *2^-8, DoubleRow pair layout [128,(c//128),9,Cout]
    w8 = np.ascontiguousarray(
        (w_t * 2.0 ** -LO_SHIFT).reshape(2, 128, 9, C_OUT).transpose(1, 0, 2, 3)
    ).astype(_E4M3)

    # --- BN + PACT constant folding (reference fp32 semantics first) ---
    g = np.asarray(gamma, dtype=np.float32)
    bt = np.asarray(beta, dtype=np.float32)
    rm = np.asarray(running_mean, dtype=np.float32)
    rv = np.asarray(running_var, dtype=np.float32)
    a = np.float32(np.asarray(alpha).reshape(-1)[0])
    inv = g / np.sqrt(rv + np.float32(1e-5))
    bb = bt - rm * inv
    # psum = 15*conv(x,wq); z = (psum/15*inv + bb)*(15/a) = psum*(inv/a) + bb*(15/a)
    scale_c = (inv.astype(np.float64) / np.float64(a)).astype(np.float32)
    bias_c = (bb.astype(np.float64) * (15.0 / np.float64(a))).astype(np.float32)
    fscale = np.full(C_OUT, a / np.float32(15.0), dtype=np.float32)
    cs = np.zeros((128, MT, 4), dtype=np.float32)
    for m in range(MT):
        cs[:, m, 0] = scale_c[m * 128:(m + 1) * 128]
        cs[:, m, 1] = bias_c[m * 128:(m + 1) * 128]
        cs[:, m, 2] = fscale[m * 128:(m + 1) * 128]

    # --- x: channel-major, zero-padded halo, bf16 hi + fp8 lo split ---
    x32 = np.asarray(x, dtype=np.float32)
    xp = np.zeros((C_IN, B, HP, WP), dtype=np.float32)
    xp[:, :, 1:1 + H, 1:1 + W] = x32.transpose(1, 0, 2, 3)
    xh = xp.astype(_BF16)
    xl = ((xp - xh.astype(np.float32)) * 2.0 ** LO_SHIFT).astype(_E4M3)
    xh = xh.reshape(KT, 128, B, HP, WP)
    xl = np.ascontiguousarray(
        xl.reshape(2, 128, B, HP, WP).transpose(1, 0, 2, 3, 4))

    in_maps = []
    for i in range(N_CORES):
        sl = slice(i * B_LOC, (i + 1) * B_LOC)
        in_maps.append({
            "xh": np.ascontiguousarray(xh[:, :, sl]),
            "xl": np.ascontiguousarray(xl[:, :, sl]),
            "wq": wq,
            "w8": w8,
            "cs": cs,
        })
    return in_maps


def kernel(x, weight, gamma, beta, running_mean, running_var, alpha):
    from concourse.bass_utils import run_bass_kernel_spmd

    nc = _get_nc()
    in_maps = _prep_inputs(x, weight, gamma, beta, running_mean, running_var, alpha)
    res = run_bass_kernel_spmd(nc, in_maps, list(range(N_CORES)))
    outs = [r["out"].reshape(B_LOC, C_OUT, H, W) for r in res.results]
    return np.concatenate(outs, axis=0)
